# revision 8
# baseline (speedup 1.0000x reference)
"""Multi-head attention (B=16, N=1024, D=512, H=8) on 8 TRN2 NeuronCores. v3.

Strategy: pure data-parallel over batch (2 batches/core, no collectives).
All input transposes (x.T, W.T, mask.T) are done host-side in numpy.

v3 vs v2 (cost-model-guided rebalance; v2 was PE-bound at 223us busy):
  - QK matmuls write BF16 PSUM E^T tiles; a group of 2 k-tiles x 2 heads
    fits 2 banks, so exp runs at FD=2048 (ACT exp 173us -> ~118us).
  - softmax denominator off PE: DVE tree-sums the exp groups (4 bf16-2x
    adds/iter), PE does only 2 final ones-matmuls per iter into dnb
    (PE dn cost 54.6us -> 6.8us).
  - bp bias + out-proj PSUM->SBUF evacuation fused into one DVE
    tensor_tensor add (drops v2's K=1 ones-seed matmuls from PE).
Engine busy targets (sim): PE ~171us, DVE ~171us, ACT ~146us.

On-device, per core:
  phase 1: Q^T,K^T [channel, token] and V [token, channel] projections (bf16)
  phase 2: k-major attention per (head-pair, batch, q-half):
           4 groups of (2 kt x 2 heads): E^T bf16 PSUM via 4 matmuls (K=64),
           exp FD=2048 on ACT (scale=1/8, no max-subtraction; logits ~N(0,1)),
           mask multiply on DVE (bf16 2x, mask broadcast across head axis),
           PV matmuls (trailing one group) accumulate head_out^T in f32 PSUM;
           dn: DVE adds g0+g1, g2+g3, Sa+Sb, kt-fold; 2 ones-matmuls -> dnb;
           epilogue: reciprocal + fused normalize into A^T buffer
  phase 3: output projection from A^T; DVE evacuates PSUM with fused +bp;
           DMA out [n,o] row-major.
"""

import os

import numpy as np
import ml_dtypes

import concourse.bass as bass
import concourse.mybir as mybir
import concourse.tile as tile
from concourse.bass_utils import run_bass_kernel_spmd
from concourse.vector_clock import ScopedClock

# ---------------------------------------------------------------------------
# Workaround: the walrus build in this container only supports ONE sync-wait
# command per instruction, but this Tile snapshot emits instructions carrying
# several.  Split surplus waits onto preceding same-engine
# InstEventSemaphore carriers (semantically identical; engine blocks on each
# wait in stream order).
# ---------------------------------------------------------------------------
_orig_commit = tile.TileContext._commit_instruction


def _split_commit(self, inst, lazy_reg_writes=True):
    si = inst.sync_info
    if (si is not None and si.on_wait and len(si.on_wait) > 1
            and inst.engine != mybir.EngineType.Unassigned):
        waits = list(si.on_wait)
        for w in waits[:-1]:
            es = mybir.InstEventSemaphore(
                name=self.nc.get_next_instruction_name(),
                engine=inst.engine, ins=[], outs=[],
                sync_info=mybir.SyncInfo(on_wait=[w], on_update=[]),
            )
            _orig_commit(self, es, lazy_reg_writes=False)
        inst.sync_info = mybir.SyncInfo(
            on_wait=[waits[-1]], on_update=list(si.on_update or []))
    return _orig_commit(self, inst, lazy_reg_writes)


def _patched_drain_and_barrier(self, tick_clock, wait_clock):
    drain_inst = self.nc.sync.drain()
    wait_clock.add_sem_waits(
        drain_inst.ins, ScopedClock({None: tick_clock.global_clock}))
    si = drain_inst.ins.sync_info
    if si is not None and si.on_wait and len(si.on_wait) > 1:
        waits = list(si.on_wait)
        drain_inst.ins.sync_info = mybir.SyncInfo(
            on_wait=[waits[0]], on_update=list(si.on_update or []))
        for w in waits[1:]:
            es = mybir.InstEventSemaphore(
                name=self.nc.get_next_instruction_name(),
                engine=mybir.EngineType.SP, ins=[], outs=[],
                sync_info=mybir.SyncInfo(on_wait=[w], on_update=[]),
            )
            self._add_instruction(es)
    self.nc.all_engine_barrier()
    assert self.sems is not None
    popped = self.nc._tile_sem_poison_stack.pop()
    assert popped is self._sem_poison
    self.nc.clear_and_free_semaphores(list(self.sems.allocated().values()))
    self.nc.all_engine_barrier()


tile.TileContext._commit_instruction = _split_commit
tile.TileContext._drain_and_barrier = _patched_drain_and_barrier

P = 128
NB = 2            # batches per core
N = 1024          # sequence length
D = 512           # model dim
H = 8             # heads
HD = 64           # head dim
T = NB * N        # tokens per core
DC = D // P       # channel chunks (4)
KT = N // P       # k tiles per batch (8)
TC = T // P       # token chunks (16)
NG = 4            # kt-groups per (hp, b, qh): 2 kt each
NCORES = 8

BF = mybir.dt.bfloat16
F32 = mybir.dt.float32
AF = mybir.ActivationFunctionType
ALU = mybir.AluOpType

_cache = {}


def _build(reps=1):
    nc = bass.Bass()

    xT_d = nc.declare_dram_parameter("xT", [D, T], BF, isOutput=False)
    wq_d = nc.declare_dram_parameter("WqT", [D, D], BF, isOutput=False)
    wk_d = nc.declare_dram_parameter("WkT", [D, D], BF, isOutput=False)
    wv_d = nc.declare_dram_parameter("WvT", [D, D], BF, isOutput=False)
    wp_d = nc.declare_dram_parameter("WpT", [D, D], BF, isOutput=False)
    mk_d = nc.declare_dram_parameter("maskT", [N, N], BF, isOutput=False)
    bq_d = nc.declare_dram_parameter("bq2", [P, DC], F32, isOutput=False)
    bk_d = nc.declare_dram_parameter("bk2", [P, DC], F32, isOutput=False)
    bv_d = nc.declare_dram_parameter("bv_rep", [P, D], BF, isOutput=False)
    bp_d = nc.declare_dram_parameter("bp_rep", [P, D], BF, isOutput=False)
    out_d = nc.declare_dram_parameter("out", [NB, N, D], F32, isOutput=True)

    with tile.TileContext(nc) as tc:
        with tc.tile_pool(name="const", bufs=1) as const:
            # resident inputs, ordered so PE can start ASAP: small weights
            # first, then xT in 4 token chunks (projections consume them in
            # token order), mask before the first attention needs it.
            wq_sb = const.tile([P, DC, D], BF)
            nc.sync.dma_start(wq_sb, wq_d[:].rearrange("(c p) o -> p c o", p=P))
            wv_sb = const.tile([P, DC, D], BF)
            nc.sync.dma_start(wv_sb, wv_d[:].rearrange("(c p) o -> p c o", p=P))
            wk_sb = const.tile([P, DC, D], BF)
            nc.sync.dma_start(wk_sb, wk_d[:].rearrange("(c p) o -> p c o", p=P))
            bq_sb = const.tile([P, DC], F32)
            nc.sync.dma_start(bq_sb, bq_d[:])
            bk_sb = const.tile([P, DC], F32)
            nc.sync.dma_start(bk_sb, bk_d[:])
            xT_sb = const.tile([P, DC, T], BF)
            for tch in range(4):
                nc.sync.dma_start(
                    xT_sb[:, :, tch * 512:(tch + 1) * 512],
                    xT_d[:, tch * 512:(tch + 1) * 512].rearrange(
                        "(c p) t -> p c t", p=P))
            maskT_sb = const.tile([P, KT, N], BF)
            nc.sync.dma_start(maskT_sb, mk_d[:].rearrange("(k p) q -> p k q", p=P))
            wp_sb = const.tile([P, DC, D], BF)
            nc.sync.dma_start(wp_sb, wp_d[:].rearrange("(c p) o -> p c o", p=P))
            bv_sb = const.tile([P, D], BF)
            nc.sync.dma_start(bv_sb, bv_d[:])
            bp_sb = const.tile([P, D], BF)
            nc.sync.dma_start(bp_sb, bp_d[:])

            ones64 = const.tile([P, HD], BF)
            nc.any.memset(ones64, 1.0)

            # resident intermediates
            QT_sb = const.tile([P, DC, T], BF)   # [chan, oc, token]
            KT_sb = const.tile([P, DC, T], BF)
            V_sb = const.tile([P, TC, D], BF)    # [token, tc, chan]
            A_sb = const.tile([P, NB * DC, N], BF)  # concat-head out^T per b

            def emit_phases():
                with (
                    tc.tile_pool(name="pr_ps", bufs=2, space="PSUM") as pr_ps,
                    tc.tile_pool(name="et_ps", bufs=2, space="PSUM") as et_ps,
                    tc.tile_pool(name="pv_ps", bufs=1, space="PSUM") as pv_ps,
                    tc.tile_pool(name="dn_ps", bufs=1, space="PSUM") as dn_ps,
                    tc.tile_pool(name="ex_sb", bufs=4) as ex_pool,
                    tc.tile_pool(name="mk_sb", bufs=3) as mk_pool,
                    tc.tile_pool(name="s_sb", bufs=4) as s_pool,
                    tc.tile_pool(name="sb2", bufs=4) as sb2,
                ):
                    def emit_v_proj(b):
                        for t16 in range(b * KT, (b + 1) * KT):
                            ps = pr_ps.tile([P, D], F32, name="prproj")
                            for ic in range(DC):
                                nc.tensor.matmul(
                                    ps,
                                    lhsT=xT_sb[:, ic, t16 * P:(t16 + 1) * P],
                                    rhs=wv_sb[:, ic, :],
                                    start=(ic == 0),
                                    stop=(ic == DC - 1),
                                )
                            nc.scalar.copy(V_sb[:, t16, :], ps)
                        # bv added once per batch, on the otherwise-idle
                        # GPSIMD engine (DVE has no spare capacity)
                        nc.gpsimd.tensor_tensor(
                            V_sb[:, b * KT:(b + 1) * KT, :],
                            V_sb[:, b * KT:(b + 1) * KT, :],
                            bv_sb[:, None, :].to_broadcast((P, KT, D)),
                            ALU.add)

                    def emit_qk_proj(oc):
                        for w_sb, b_sb, dst in ((wq_sb, bq_sb, QT_sb),
                                                (wk_sb, bk_sb, KT_sb)):
                            for ns in range(T // 512):
                                ps = pr_ps.tile([P, D], F32, name="prproj")
                                for ic in range(DC):
                                    nc.tensor.matmul(
                                        ps,
                                        lhsT=w_sb[:, ic, oc * P:(oc + 1) * P],
                                        rhs=xT_sb[:, ic, ns * 512:(ns + 1) * 512],
                                        start=(ic == 0),
                                        stop=(ic == DC - 1),
                                    )
                                if dst is QT_sb:
                                    nc.scalar.activation(
                                        dst[:, oc, ns * 512:(ns + 1) * 512],
                                        ps,
                                        AF.Identity,
                                        bias=b_sb[:, oc:oc + 1],
                                    )
                                else:
                                    # K copies on DVE to balance ACT
                                    nc.vector.tensor_scalar_add(
                                        dst[:, oc, ns * 512:(ns + 1) * 512],
                                        ps,
                                        b_sb[:, oc:oc + 1],
                                    )

                    def emit_qk_group(b, hp, qh, g):
                        # E^T for 2 kt x 2 heads; f32 PSUM per kt (2 banks),
                        # both kt's exps land in one bf16 SBUF group tile so
                        # the mask multiply and dn adds run at FD=2048.
                        ex = ex_pool.tile([P, 2, 2, 512], BF, name="ex")
                        for kti in range(2):
                            kt = 2 * g + kti
                            et = et_ps.tile([P, 2, 512], F32, name="et")
                            for sub in range(2):
                                po = sub * HD
                                nc.tensor.matmul(
                                    et[:, sub, :],
                                    lhsT=KT_sb[po:po + HD, hp,
                                               b * N + kt * P: b * N + (kt + 1) * P],
                                    rhs=QT_sb[po:po + HD, hp,
                                              b * N + qh * 512: b * N + (qh + 1) * 512],
                                    start=True,
                                    stop=True,
                                )
                            nc.scalar.activation(
                                ex[:, kti, :, :], et, AF.Exp, scale=0.125)
                        return ex

                    def emit_mask(ex, qh, g):
                        # one FD=2048 mask mult per group, mask rows broadcast
                        # across the head axis.
                        mk = mk_pool.tile([P, 2, 2, 512], BF, name="mk")
                        nc.vector.tensor_tensor(
                            mk, ex,
                            maskT_sb[:, 2 * g:2 * g + 2, None,
                                     qh * 512:(qh + 1) * 512].to_broadcast(
                                         (P, 2, 2, 512)),
                            ALU.mult)
                        return mk

                    def emit_pv_group(b, hp, g, pv, mk):
                        for kti in range(2):
                            kt = 2 * g + kti
                            for sub in (1, 0):
                                po = sub * HD
                                nc.tensor.matmul(
                                    pv[po:po + HD, :],
                                    lhsT=V_sb[:, b * KT + kt,
                                              (2 * hp + sub) * HD:(2 * hp + sub + 1) * HD],
                                    rhs=mk[:, kti, sub, :],
                                    start=(kt == 0),
                                    stop=(kt == KT - 1),
                                    tile_position=(0, po),
                                )

                    def emit_proj_quarter(b, qh):
                        # output projection quarter; PSUM evacuated by DVE
                        # with the +bp broadcast add fused in.
                        for nt in range(qh * 4, (qh + 1) * 4):
                            ps = pr_ps.tile([P, 512], F32, name="prproj")
                            for cc in range(DC):
                                nc.tensor.matmul(
                                    ps,
                                    lhsT=A_sb[:, b * DC + cc, nt * P:(nt + 1) * P],
                                    rhs=wp_sb[:, cc, :],
                                    start=(cc == 0),
                                    stop=(cc == DC - 1),
                                )
                            fo = sb2.tile([P, 512], F32, name="rb", tag="rb")
                            nc.vector.tensor_tensor(
                                fo, ps, bp_sb, ALU.add)
                            nc.sync.dma_start(out_d[b, nt * P:(nt + 1) * P, :],
                                              fo)

                    def emit_attention(hp, b, qh, fin_prev):
                        pv = pv_ps.tile([P, 512], F32, name="pv")
                        # software pipeline: PV trails QK/exp/mask by ONE
                        # group; dn is a per-group kt-fold h_g (g0-2 on the
                        # idle GPSIMD, g3 on DVE for latency) + a running DVE
                        # chain; the finalize (dnb matmuls, recip, normalize)
                        # is deferred into the NEXT iteration's stream so the
                        # PE never waits on the DVE chain.
                        hs = []
                        acc = None
                        pend = []
                        for g in range(NG):
                            ex = emit_qk_group(b, hp, qh, g)
                            mk = emit_mask(ex, qh, g)
                            pend.append((g, mk))
                            h = s_pool.tile([P, 2, 512], BF, name="h")
                            eng = nc.vector if g == NG - 1 else nc.gpsimd
                            eng.tensor_tensor(
                                h, ex[:, 0, :, :], ex[:, 1, :, :], ALU.add)
                            hs.append(h)
                            if g == 0 and fin_prev is not None:
                                fin_prev()
                            if len(pend) > 1:
                                g0, mk0 = pend.pop(0)
                                emit_pv_group(b, hp, g0, pv, mk0)
                            if g >= 1:
                                # chain adds for g1/g2 have >=1 group of
                                # slack -> idle GPSIMD; g3 is on the tail
                                # latency path -> DVE
                                nxt = s_pool.tile([P, 2, 512], BF, name="hacc")
                                eng2 = nc.vector if g == NG - 1 else nc.gpsimd
                                eng2.tensor_tensor(
                                    nxt, acc if acc is not None else hs[0],
                                    hs[g], ALU.add)
                                acc = nxt
                        for g0, mk0 in pend:
                            emit_pv_group(b, hp, g0, pv, mk0)

                        def fin(acc=acc, pv=pv, hp=hp, b=b, qh=qh):
                            dnb = dn_ps.tile([P, 512], F32, name="dnb")
                            for sub in (1, 0):
                                po = sub * HD
                                nc.tensor.matmul(
                                    dnb[po:po + HD, :],
                                    lhsT=ones64,
                                    rhs=acc[:, sub, :],
                                    start=True,
                                    stop=True,
                                    tile_position=(0, po),
                                )
                            rb = sb2.tile([P, 512], F32, name="rb")
                            nc.vector.reciprocal(rb, dnb)
                            nc.vector.tensor_tensor(
                                A_sb[:, b * DC + hp, qh * 512:(qh + 1) * 512],
                                pv, rb, ALU.mult
                            )
                        return fin

                    fin = None
                    for oc in range(DC):
                        emit_qk_proj(oc)
                        if oc == 0:
                            emit_v_proj(0)
                        for b in range(NB):
                            if oc == 0 and b == 1:
                                emit_v_proj(1)
                            for qh in range(2):
                                fin = emit_attention(oc, b, qh, fin)
                                if oc == DC - 1:
                                    fin()
                                    fin = None
                                    emit_proj_quarter(b, qh)

            if reps == 1:
                emit_phases()
            else:
                with tc.For_i(0, reps, 1):
                    emit_phases()

    return nc


def make_in_maps(inputs):
    bf = ml_dtypes.bfloat16
    x = np.asarray(inputs["x"], np.float32)
    mask = np.asarray(inputs["mask"], np.float32)
    shared = {
        "WqT": np.ascontiguousarray(np.asarray(inputs["Wq"], np.float32).T).astype(bf),
        "WkT": np.ascontiguousarray(np.asarray(inputs["Wk"], np.float32).T).astype(bf),
        "WvT": np.ascontiguousarray(np.asarray(inputs["Wv"], np.float32).T).astype(bf),
        "WpT": np.ascontiguousarray(np.asarray(inputs["Wp"], np.float32).T).astype(bf),
        "maskT": np.ascontiguousarray(mask.T).astype(bf),
        "bq2": np.ascontiguousarray(np.asarray(inputs["bq"], np.float32).reshape(DC, P).T),
        "bk2": np.ascontiguousarray(np.asarray(inputs["bk"], np.float32).reshape(DC, P).T),
        "bv_rep": np.tile(np.asarray(inputs["bv"], np.float32).astype(bf)[None, :], (P, 1)),
        "bp_rep": np.tile(np.asarray(inputs["bp"], np.float32).astype(bf)[None, :], (P, 1)),
    }
    in_maps = []
    for c in range(NCORES):
        xT = np.ascontiguousarray(
            x[NB * c: NB * (c + 1)].reshape(T, D).T
        ).astype(bf)
        in_maps.append({"xT": xT, **shared})
    return in_maps


def kernel(x, mask, Wq, bq, Wk, bk, Wv, bv, Wp, bp):
    if "nc" not in _cache:
        _cache["nc"] = _build()
    nc = _cache["nc"]

    in_maps = make_in_maps(dict(x=x, mask=mask, Wq=Wq, bq=bq, Wk=Wk, bk=bk,
                                Wv=Wv, bv=bv, Wp=Wp, bp=bp))

    trace_dir = os.environ.get("BASS_TRACE_DIR")
    if trace_dir:
        import concourse.bass_utils as bu
        bu.upload_artifacts = lambda tmpdir: "local"
        res = run_bass_kernel_spmd(
            nc, in_maps, core_ids=list(range(NCORES)), trace=True,
            tmpdir=trace_dir,
        )
        kernel.last_exec_time_ns = res.exec_time_ns
        kernel.last_results = res
    else:
        res = run_bass_kernel_spmd(nc, in_maps, core_ids=list(range(NCORES)))

    outs = [np.asarray(r["out"], np.float32) for r in res.results]
    return np.concatenate(outs, axis=0)


# revision 14
# speedup vs baseline: 1.2331x; 1.2331x over previous
"""Multi-head attention (B=16, N=1024, D=512, H=8) on 8 TRN2 NeuronCores. v3.

Strategy: pure data-parallel over batch (2 batches/core, no collectives).
All input transposes (x.T, W.T, mask.T) are done host-side in numpy.

v3 vs v2 (cost-model-guided rebalance; v2 was PE-bound at 223us busy):
  - QK matmuls write BF16 PSUM E^T tiles; a group of 2 k-tiles x 2 heads
    fits 2 banks, so exp runs at FD=2048 (ACT exp 173us -> ~118us).
  - softmax denominator off PE: DVE tree-sums the exp groups (4 bf16-2x
    adds/iter), PE does only 2 final ones-matmuls per iter into dnb
    (PE dn cost 54.6us -> 6.8us).
  - bp bias + out-proj PSUM->SBUF evacuation fused into one DVE
    tensor_tensor add (drops v2's K=1 ones-seed matmuls from PE).
Engine busy targets (sim): PE ~171us, DVE ~171us, ACT ~146us.

On-device, per core:
  phase 1: Q^T,K^T [channel, token] and V [token, channel] projections (bf16)
  phase 2: k-major attention per (head-pair, batch, q-half):
           4 groups of (2 kt x 2 heads): E^T bf16 PSUM via 4 matmuls (K=64),
           exp FD=2048 on ACT (scale=1/8, no max-subtraction; logits ~N(0,1)),
           mask multiply on DVE (bf16 2x, mask broadcast across head axis),
           PV matmuls (trailing one group) accumulate head_out^T in f32 PSUM;
           dn: DVE adds g0+g1, g2+g3, Sa+Sb, kt-fold; 2 ones-matmuls -> dnb;
           epilogue: reciprocal + fused normalize into A^T buffer
  phase 3: output projection from A^T; DVE evacuates PSUM with fused +bp;
           DMA out [n,o] row-major.
"""

import os

import numpy as np
import ml_dtypes

import concourse.bass as bass
import concourse.mybir as mybir
import concourse.tile as tile
from concourse.bass_utils import run_bass_kernel_spmd
from concourse.vector_clock import ScopedClock

# ---------------------------------------------------------------------------
# Workaround: the walrus build in this container only supports ONE sync-wait
# command per instruction, but this Tile snapshot emits instructions carrying
# several.  Split surplus waits onto preceding same-engine
# InstEventSemaphore carriers (semantically identical; engine blocks on each
# wait in stream order).
# ---------------------------------------------------------------------------
_orig_commit = tile.TileContext._commit_instruction


def _split_commit(self, inst, lazy_reg_writes=True):
    si = inst.sync_info
    if (si is not None and si.on_wait and len(si.on_wait) > 1
            and inst.engine != mybir.EngineType.Unassigned):
        waits = list(si.on_wait)
        for w in waits[:-1]:
            es = mybir.InstEventSemaphore(
                name=self.nc.get_next_instruction_name(),
                engine=inst.engine, ins=[], outs=[],
                sync_info=mybir.SyncInfo(on_wait=[w], on_update=[]),
            )
            _orig_commit(self, es, lazy_reg_writes=False)
        inst.sync_info = mybir.SyncInfo(
            on_wait=[waits[-1]], on_update=list(si.on_update or []))
    return _orig_commit(self, inst, lazy_reg_writes)


def _patched_drain_and_barrier(self, tick_clock, wait_clock):
    drain_inst = self.nc.sync.drain()
    wait_clock.add_sem_waits(
        drain_inst.ins, ScopedClock({None: tick_clock.global_clock}))
    si = drain_inst.ins.sync_info
    if si is not None and si.on_wait and len(si.on_wait) > 1:
        waits = list(si.on_wait)
        drain_inst.ins.sync_info = mybir.SyncInfo(
            on_wait=[waits[0]], on_update=list(si.on_update or []))
        for w in waits[1:]:
            es = mybir.InstEventSemaphore(
                name=self.nc.get_next_instruction_name(),
                engine=mybir.EngineType.SP, ins=[], outs=[],
                sync_info=mybir.SyncInfo(on_wait=[w], on_update=[]),
            )
            self._add_instruction(es)
    self.nc.all_engine_barrier()
    assert self.sems is not None
    popped = self.nc._tile_sem_poison_stack.pop()
    assert popped is self._sem_poison
    self.nc.clear_and_free_semaphores(list(self.sems.allocated().values()))
    self.nc.all_engine_barrier()


tile.TileContext._commit_instruction = _split_commit
tile.TileContext._drain_and_barrier = _patched_drain_and_barrier

P = 128
NB = 2            # batches per core
N = 1024          # sequence length
D = 512           # model dim
H = 8             # heads
HD = 64           # head dim
T = NB * N        # tokens per core
DC = D // P       # channel chunks (4)
KT = N // P       # k tiles per batch (8)
TC = T // P       # token chunks (16)
NG = 4            # kt-groups per (hp, b, qh): 2 kt each
NCORES = 8

BF = mybir.dt.bfloat16
F32 = mybir.dt.float32
AF = mybir.ActivationFunctionType
ALU = mybir.AluOpType

_cache = {}


def _build(reps=1):
    nc = bass.Bass()

    xT_d = nc.declare_dram_parameter("xT", [D, T], BF, isOutput=False)
    wq_d = nc.declare_dram_parameter("WqT", [D, D], BF, isOutput=False)
    wk_d = nc.declare_dram_parameter("WkT", [D, D], BF, isOutput=False)
    wv_d = nc.declare_dram_parameter("WvT", [D, D], BF, isOutput=False)
    wp_d = nc.declare_dram_parameter("WpT", [D, D], BF, isOutput=False)
    mk_d = nc.declare_dram_parameter("maskT", [N, N], BF, isOutput=False)
    bq_d = nc.declare_dram_parameter("bq2", [P, DC], F32, isOutput=False)
    bk_d = nc.declare_dram_parameter("bk2", [P, DC], F32, isOutput=False)
    bv_d = nc.declare_dram_parameter("bv_rep", [P, D], BF, isOutput=False)
    bp_d = nc.declare_dram_parameter("bp_rep", [P, D], BF, isOutput=False)
    out_d = nc.declare_dram_parameter("out", [NB, N, D], F32, isOutput=True)

    with tile.TileContext(nc) as tc:
        with tc.tile_pool(name="const", bufs=1) as const:
            # resident inputs, ordered so PE can start ASAP: small weights
            # first, then xT in 4 token chunks (projections consume them in
            # token order), mask before the first attention needs it.
            wq_sb = const.tile([P, DC, D], BF)
            nc.sync.dma_start(wq_sb, wq_d[:].rearrange("(c p) o -> p c o", p=P))
            wv_sb = const.tile([P, DC, D], BF)
            nc.sync.dma_start(wv_sb, wv_d[:].rearrange("(c p) o -> p c o", p=P))
            wk_sb = const.tile([P, DC, D], BF)
            nc.sync.dma_start(wk_sb, wk_d[:].rearrange("(c p) o -> p c o", p=P))
            bq_sb = const.tile([P, DC], F32)
            nc.sync.dma_start(bq_sb, bq_d[:])
            bk_sb = const.tile([P, DC], F32)
            nc.sync.dma_start(bk_sb, bk_d[:])
            xT_sb = const.tile([P, DC, T], BF)
            for tch in range(4):
                nc.sync.dma_start(
                    xT_sb[:, :, tch * 512:(tch + 1) * 512],
                    xT_d[:, tch * 512:(tch + 1) * 512].rearrange(
                        "(c p) t -> p c t", p=P))
            maskT_sb = const.tile([P, KT, N], BF)
            nc.sync.dma_start(maskT_sb, mk_d[:].rearrange("(k p) q -> p k q", p=P))
            wp_sb = const.tile([P, DC, D], BF)
            nc.sync.dma_start(wp_sb, wp_d[:].rearrange("(c p) o -> p c o", p=P))
            bv_sb = const.tile([P, D], BF)
            nc.sync.dma_start(bv_sb, bv_d[:])
            bp_sb = const.tile([P, D], BF)
            nc.sync.dma_start(bp_sb, bp_d[:])

            ones64 = const.tile([P, HD], BF)
            nc.any.memset(ones64, 1.0)

            # resident intermediates
            QT_sb = const.tile([P, DC, T], BF)   # [chan, oc, token]
            KT_sb = const.tile([P, DC, T], BF)
            V_sb = const.tile([P, TC, D], BF)    # [token, tc, chan]
            A_sb = const.tile([P, NB * DC, N], BF)  # concat-head out^T per b

            def emit_phases():
                with (
                    tc.tile_pool(name="pr_ps", bufs=2, space="PSUM") as pr_ps,
                    tc.tile_pool(name="et_ps", bufs=2, space="PSUM") as et_ps,
                    tc.tile_pool(name="pv_ps", bufs=1, space="PSUM") as pv_ps,
                    tc.tile_pool(name="dn_ps", bufs=1, space="PSUM") as dn_ps,
                    tc.tile_pool(name="ex_sb", bufs=5) as ex_pool,
                    tc.tile_pool(name="mk_sb", bufs=4) as mk_pool,
                    tc.tile_pool(name="s_sb", bufs=3) as s_pool,
                    tc.tile_pool(name="sb2", bufs=6) as sb2,
                ):
                    def emit_v_tile(b, t16):
                        ps = pr_ps.tile([P, D], F32, name="prproj")
                        for ic in range(DC):
                            nc.tensor.matmul(
                                ps,
                                lhsT=xT_sb[:, ic, t16 * P:(t16 + 1) * P],
                                rhs=wv_sb[:, ic, :],
                                start=(ic == 0),
                                stop=(ic == DC - 1),
                            )
                        nc.scalar.copy(V_sb[:, t16, :], ps)
                        if t16 == (b + 1) * KT - 1:
                            # bv added once per batch on DVE
                            nc.vector.tensor_tensor(
                                V_sb[:, b * KT:(b + 1) * KT, :],
                                V_sb[:, b * KT:(b + 1) * KT, :],
                                bv_sb[:, None, :].to_broadcast((P, KT, D)),
                                ALU.add)

                    def emit_v_proj(b):
                        for t16 in range(b * KT, (b + 1) * KT):
                            emit_v_tile(b, t16)

                    def emit_qk_tile(oc, which, ns):
                        w_sb, b_sb, dst = (
                            (wq_sb, bq_sb, QT_sb) if which == 0
                            else (wk_sb, bk_sb, KT_sb))
                        ps = pr_ps.tile([P, D], F32, name="prproj")
                        for ic in range(DC):
                            nc.tensor.matmul(
                                ps,
                                lhsT=w_sb[:, ic, oc * P:(oc + 1) * P],
                                rhs=xT_sb[:, ic, ns * 512:(ns + 1) * 512],
                                start=(ic == 0),
                                stop=(ic == DC - 1),
                            )
                        nc.scalar.activation(
                            dst[:, oc, ns * 512:(ns + 1) * 512],
                            ps,
                            AF.Identity,
                            bias=b_sb[:, oc:oc + 1],
                        )

                    def emit_qk_proj(oc):
                        for which in range(2):
                            for ns in range(T // 512):
                                emit_qk_tile(oc, which, ns)

                    def emit_qk_group(b, hp, qh, g):
                        # E^T for 2 kt x 2 heads; f32 PSUM per kt (2 banks),
                        # both kt's exps land in one bf16 SBUF group tile so
                        # the mask multiply and dn adds run at FD=2048.
                        ex = ex_pool.tile([P, 2, 2, 512], BF, name="ex")
                        for kti in range(2):
                            kt = 2 * g + kti
                            et = et_ps.tile([P, 2, 512], F32, name="et")
                            for sub in range(2):
                                po = sub * HD
                                nc.tensor.matmul(
                                    et[:, sub, :],
                                    lhsT=KT_sb[po:po + HD, hp,
                                               b * N + kt * P: b * N + (kt + 1) * P],
                                    rhs=QT_sb[po:po + HD, hp,
                                              b * N + qh * 512: b * N + (qh + 1) * 512],
                                    start=True,
                                    stop=True,
                                )
                            nc.scalar.activation(
                                ex[:, kti, :, :], et, AF.Exp, scale=0.125)
                        return ex

                    def emit_mask(ex, qh, g):
                        # one FD=2048 mask mult per group, mask rows broadcast
                        # across the head axis.
                        mk = mk_pool.tile([P, 2, 2, 512], BF, name="mk")
                        nc.vector.tensor_tensor(
                            mk, ex,
                            maskT_sb[:, 2 * g:2 * g + 2, None,
                                     qh * 512:(qh + 1) * 512].to_broadcast(
                                         (P, 2, 2, 512)),
                            ALU.mult)
                        return mk

                    def emit_pv_group(b, hp, g, pv, mk):
                        for kti in range(2):
                            kt = 2 * g + kti
                            for sub in (1, 0):
                                po = sub * HD
                                nc.tensor.matmul(
                                    pv[po:po + HD, :],
                                    lhsT=V_sb[:, b * KT + kt,
                                              (2 * hp + sub) * HD:(2 * hp + sub + 1) * HD],
                                    rhs=mk[:, kti, sub, :],
                                    start=(kt == 0),
                                    stop=(kt == KT - 1),
                                    tile_position=(0, po),
                                )

                    def emit_proj_quarter(b, qh):
                        # output projection quarter; PSUM evacuated by DVE
                        # with the +bp broadcast add fused in.
                        for nt in range(qh * 4, (qh + 1) * 4):
                            ps = pr_ps.tile([P, 512], F32, name="prproj")
                            for cc in range(DC):
                                nc.tensor.matmul(
                                    ps,
                                    lhsT=A_sb[:, b * DC + cc, nt * P:(nt + 1) * P],
                                    rhs=wp_sb[:, cc, :],
                                    start=(cc == 0),
                                    stop=(cc == DC - 1),
                                )
                            fo = sb2.tile([P, 512], F32, name="rb", tag="rb")
                            nc.vector.tensor_tensor(
                                fo, ps, bp_sb, ALU.add)
                            nc.sync.dma_start(out_d[b, nt * P:(nt + 1) * P, :],
                                              fo)

                    def emit_attention(hp, b, qh, fin_prev, feed=None):
                        pv = pv_ps.tile([P, 512], F32, name="pv")
                        # software pipeline: PV trails QK/exp/mask by ONE
                        # group; dn is a per-group kt-fold h_g (g0-2 on the
                        # idle GPSIMD, g3 on DVE for latency) + a running DVE
                        # chain; the finalize (dnb matmuls, recip, normalize)
                        # is deferred into the NEXT iteration's stream so the
                        # PE never waits on the DVE chain.
                        exs = []
                        pend = []
                        sab = []
                        for g in range(NG):
                            ex = emit_qk_group(b, hp, qh, g)
                            mk = emit_mask(ex, qh, g)
                            exs.append(ex)
                            pend.append((g, mk))
                            if g == 0 and fin_prev is not None:
                                fin_prev()
                            if feed is not None and g in (1, 3):
                                feed(g)
                            if len(pend) > 1:
                                g0, mk0 = pend.pop(0)
                                emit_pv_group(b, hp, g0, pv, mk0)
                            if g in (1, 3):
                                st = s_pool.tile([P, 2, 2, 512], BF, name="st")
                                nc.vector.tensor_tensor(
                                    st, exs[g - 1], exs[g], ALU.add)
                                sab.append(st)
                        # S/Sf folds happen in THIS iteration (right after
                        # Sb) so the deferred dn-final matmuls never block
                        # the next iteration's PE stream.
                        s = s_pool.tile([P, 2, 2, 512], BF, name="s")
                        nc.vector.tensor_tensor(s, sab[0], sab[1], ALU.add)
                        sf = s_pool.tile([P, 2, 512], BF, name="sf")
                        nc.vector.tensor_tensor(
                            sf, s[:, 0, :, :], s[:, 1, :, :], ALU.add)
                        for g0, mk0 in pend:
                            emit_pv_group(b, hp, g0, pv, mk0)

                        def fin(sf=sf, pv=pv, hp=hp, b=b, qh=qh):
                            dnb = dn_ps.tile([P, 512], F32, name="dnb")
                            for sub in (1, 0):
                                po = sub * HD
                                nc.tensor.matmul(
                                    dnb[po:po + HD, :],
                                    lhsT=ones64,
                                    rhs=sf[:, sub, :],
                                    start=True,
                                    stop=True,
                                    tile_position=(0, po),
                                )
                            rb = sb2.tile([P, 512], F32, name="rb")
                            nc.vector.reciprocal(rb, dnb)
                            nc.vector.tensor_tensor(
                                A_sb[:, b * DC + hp, qh * 512:(qh + 1) * 512],
                                pv, rb, ALU.mult
                            )
                        return fin

                    import collections
                    feed_q = collections.deque()
                    for t16 in range(KT, TC):
                        feed_q.append(lambda t16=t16: emit_v_tile(1, t16))
                    for oc in range(1, DC):
                        for which, ns in ((0, 0), (1, 0), (0, 1), (1, 1),
                                          (0, 2), (1, 2), (0, 3), (1, 3)):
                            feed_q.append(
                                lambda oc=oc, w=which, ns=ns:
                                emit_qk_tile(oc, w, ns))
                    # per-iteration feed budget: 4 tiles for the first 4
                    # iters (V b1 + proj oc1), 2 after; 0 during oc=3 where
                    # the out-projection quarters take the slack.
                    feed_counts = [4] * 4 + [2] * 8 + [0] * 4

                    emit_qk_proj(0)
                    emit_v_proj(0)
                    fin = None
                    it = 0
                    for oc in range(DC):
                        for b in range(NB):
                            for qh in range(2):
                                budget = feed_counts[it]

                                def feed(g, budget=budget):
                                    n = budget // 2 if g == 1 else (
                                        budget - budget // 2)
                                    for _ in range(n):
                                        if feed_q:
                                            feed_q.popleft()()
                                fin = emit_attention(oc, b, qh, fin, feed)
                                if oc == DC - 1:
                                    fin()
                                    fin = None
                                    emit_proj_quarter(b, qh)
                                it += 1
                    assert not feed_q

            if reps == 1:
                emit_phases()
            else:
                with tc.For_i(0, reps, 1):
                    emit_phases()

    return nc


def make_in_maps(inputs):
    bf = ml_dtypes.bfloat16
    x = np.asarray(inputs["x"], np.float32)
    mask = np.asarray(inputs["mask"], np.float32)
    shared = {
        "WqT": np.ascontiguousarray(np.asarray(inputs["Wq"], np.float32).T).astype(bf),
        "WkT": np.ascontiguousarray(np.asarray(inputs["Wk"], np.float32).T).astype(bf),
        "WvT": np.ascontiguousarray(np.asarray(inputs["Wv"], np.float32).T).astype(bf),
        "WpT": np.ascontiguousarray(np.asarray(inputs["Wp"], np.float32).T).astype(bf),
        "maskT": np.ascontiguousarray(mask.T).astype(bf),
        "bq2": np.ascontiguousarray(np.asarray(inputs["bq"], np.float32).reshape(DC, P).T),
        "bk2": np.ascontiguousarray(np.asarray(inputs["bk"], np.float32).reshape(DC, P).T),
        "bv_rep": np.tile(np.asarray(inputs["bv"], np.float32).astype(bf)[None, :], (P, 1)),
        "bp_rep": np.tile(np.asarray(inputs["bp"], np.float32).astype(bf)[None, :], (P, 1)),
    }
    in_maps = []
    for c in range(NCORES):
        xT = np.ascontiguousarray(
            x[NB * c: NB * (c + 1)].reshape(T, D).T
        ).astype(bf)
        in_maps.append({"xT": xT, **shared})
    return in_maps


def kernel(x, mask, Wq, bq, Wk, bk, Wv, bv, Wp, bp):
    if "nc" not in _cache:
        _cache["nc"] = _build()
    nc = _cache["nc"]

    in_maps = make_in_maps(dict(x=x, mask=mask, Wq=Wq, bq=bq, Wk=Wk, bk=bk,
                                Wv=Wv, bv=bv, Wp=Wp, bp=bp))

    trace_dir = os.environ.get("BASS_TRACE_DIR")
    if trace_dir:
        import concourse.bass_utils as bu
        bu.upload_artifacts = lambda tmpdir: "local"
        res = run_bass_kernel_spmd(
            nc, in_maps, core_ids=list(range(NCORES)), trace=True,
            tmpdir=trace_dir,
        )
        kernel.last_exec_time_ns = res.exec_time_ns
        kernel.last_results = res
    else:
        res = run_bass_kernel_spmd(nc, in_maps, core_ids=list(range(NCORES)))

    outs = [np.asarray(r["out"], np.float32) for r in res.results]
    return np.concatenate(outs, axis=0)
